# revision 6
# baseline (speedup 1.0000x reference)
"""Trainium2 Bass kernel for the KAN-to-MLP module.

Math: out = GELU( silu(x) @ base_w.T + einsum('nhk,ohk->no', bsplines(x), spline_w * scaler) )

Reformulation: both branches fuse into ONE matmul with contraction
K = H (silu branch) + 8*H (8 B-spline basis planes) = 9216 against a
host-prepacked weight Wcat (9216, 4096).  The uniform cubic B-spline
bases are computed on-device in closed form: for u = 2.5x + 2.5,
i = floor(u), t = u - i, the only nonzero bases are planes j = i..i+3
with values  [(1-t)^3/6, (3t^3-6t^2+4)/6, (-3t^3+3t^2+3t+1)/6, t^3/6].

Sharding: data-parallel over tokens (8192 rows -> 1024/core), weights
replicated.  Per core the kernel computes features in transposed
(K x token) layout, then out(d,tok) = sum_k W(k,d)^T feat(k,tok) with
W stationary on the PE and features moving, bf16 inputs with fp32 PSUM
accumulation, GELU fused on the scalar engine.
"""

import sys

for _p in ("/opt/trn_rl_repo",):
    if _p not in sys.path:
        sys.path.insert(0, _p)

import numpy as np
import ml_dtypes

import concourse.bass as bass
import concourse.tile as tile
from concourse import bacc, mybir
from concourse.bass_utils import run_bass_kernel_spmd

AF = mybir.ActivationFunctionType
ALU = mybir.AluOpType
DT = mybir.dt

N_CORES = 8
NTOK = 1024          # tokens per core
H = 1024             # input dim
D = 4096             # output dim
NB = 8               # number of basis functions
KTOT = H + NB * H    # 9216 contraction
KT = KTOT // 128     # 72 k-tiles
CHUNK = 512          # tokens per chunk
NCHUNK = NTOK // CHUNK
DTI = D // 128       # 32 d-tiles
HT = H // 128        # 8 h-tiles

_NC_CACHE = {}


def _build_program():
    nc = bacc.Bacc("TRN2", target_bir_lowering=False, debug=False,
                   enable_asserts=False, num_devices=N_CORES)
    xt = nc.dram_tensor("xt", (H, NTOK), DT.float32, kind="ExternalInput").ap()
    w = nc.dram_tensor("w", (DTI, 128, KT * 128), DT.bfloat16,
                       kind="ExternalInput").ap()
    out = nc.dram_tensor("out", (D, NTOK), DT.float32, kind="ExternalOutput").ap()

    f32 = DT.float32
    bf16 = DT.bfloat16

    with tile.TileContext(nc) as tc:
        with (
            tc.tile_pool(name="xp", bufs=2) as xp,
            tc.tile_pool(name="featp", bufs=1) as featp,
            tc.tile_pool(name="scr", bufs=2) as scr,
            tc.tile_pool(name="wp", bufs=2) as wp,
            tc.tile_pool(name="psump", bufs=4, space=bass.MemorySpace.PSUM) as psump,
            tc.tile_pool(name="outp", bufs=3) as outp,
        ):
            for c in range(NCHUNK):
                # ---- feature build: silu + 8 b-spline planes, (K x tok) ----
                feat = [featp.tile([128, CHUNK], bf16, tag=f"f{k}", name=f"feat{k}")
                        for k in range(KT)]
                for ht in range(HT):
                    xtile = xp.tile([128, CHUNK], f32, tag="x", name="xtile")
                    nc.sync.dma_start(
                        xtile[:], xt[ht * 128:(ht + 1) * 128,
                                     c * CHUNK:(c + 1) * CHUNK])
                    # silu branch -> feat[ht]
                    nc.scalar.activation(feat[ht][:], xtile[:], AF.Silu)
                    # u = 2.5x + 2.5 on ACT; clamp below 5 fused into DVE ops
                    u = scr.tile([128, CHUNK], f32, tag="u", name="u")
                    nc.scalar.activation(u[:], xtile[:], AF.Copy,
                                         bias=2.5, scale=2.5)
                    CL = 4.9999995
                    uc = scr.tile([128, CHUNK], f32, tag="uc", name="uc")
                    nc.vector.tensor_scalar_min(uc[:], u[:], CL)
                    # floor(u) for u in [0,5) as a sum of step functions
                    g = []
                    for v in range(1, 5):
                        gv = scr.tile([128, CHUNK], f32, tag=f"g{v}",
                                      name=f"g{v}")
                        nc.vector.tensor_scalar(gv[:], uc[:], float(v), None,
                                                ALU.is_ge)
                        g.append(gv)
                    ii = scr.tile([128, CHUNK], f32, tag="ii", name="ii")
                    nc.vector.tensor_add(ii[:], g[0][:], g[1][:])
                    nc.vector.tensor_add(ii[:], ii[:], g[2][:])
                    nc.vector.tensor_add(ii[:], ii[:], g[3][:])
                    t = scr.tile([128, CHUNK], f32, tag="t", name="t")
                    nc.vector.tensor_sub(t[:], uc[:], ii[:])
                    t2 = scr.tile([128, CHUNK], f32, tag="t2", name="t2")
                    nc.vector.tensor_mul(t2[:], t[:], t[:])
                    t3 = scr.tile([128, CHUNK], f32, tag="t3", name="t3")
                    nc.vector.tensor_mul(t3[:], t2[:], t[:])
                    s = scr.tile([128, CHUNK], f32, tag="s", name="s")
                    nc.vector.tensor_scalar(s[:], t[:], -1.0, 1.0,
                                            ALU.mult, ALU.add)
                    s2 = scr.tile([128, CHUNK], f32, tag="s2", name="s2")
                    nc.vector.tensor_mul(s2[:], s[:], s[:])
                    # b0 = (1-t)^3/6 ; b3 = t^3/6
                    b0 = scr.tile([128, CHUNK], f32, tag="b0", name="b0")
                    nc.vector.scalar_tensor_tensor(b0[:], s2[:], 1.0 / 6.0,
                                                   s[:], ALU.mult, ALU.mult)
                    b3 = scr.tile([128, CHUNK], f32, tag="b3", name="b3")
                    nc.vector.tensor_scalar_mul(b3[:], t3[:], 1.0 / 6.0)
                    # b1 = 0.5 t^3 - t^2 + 2/3
                    b1 = scr.tile([128, CHUNK], f32, tag="b1", name="b1")
                    nc.vector.scalar_tensor_tensor(b1[:], t3[:], 0.5, t2[:],
                                                   ALU.mult, ALU.subtract)
                    nc.vector.tensor_scalar_add(b1[:], b1[:], 2.0 / 3.0)
                    # b2 = 1 - b0 - b1 - b3
                    b2 = scr.tile([128, CHUNK], f32, tag="b2", name="b2")
                    nc.vector.tensor_add(b2[:], b0[:], b3[:])
                    nc.vector.tensor_add(b2[:], b2[:], b1[:])
                    nc.vector.tensor_scalar(b2[:], b2[:], -1.0, 1.0,
                                            ALU.mult, ALU.add)
                    bd = (b0, b1, b2, b3)
                    # plane j (basis index) = sum_d (i == j-d) * b_d
                    for j in range(NB):
                        dst = feat[HT + j * HT + ht]
                        terms = [(j - d, d) for d in range(4) if 0 <= j - d <= 4]
                        if len(terms) == 1:
                            iv, d = terms[0]
                            nc.vector.scalar_tensor_tensor(
                                dst[:], ii[:], float(iv), bd[d][:],
                                ALU.is_equal, ALU.mult)
                        else:
                            acc = scr.tile([128, CHUNK], f32, tag="acc",
                                           name="acc")
                            iv, d = terms[0]
                            nc.vector.scalar_tensor_tensor(
                                acc[:], ii[:], float(iv), bd[d][:],
                                ALU.is_equal, ALU.mult)
                            for n, (iv, d) in enumerate(terms[1:]):
                                last = n == len(terms) - 2
                                tgt = dst if last else acc
                                tmp = scr.tile([128, CHUNK], f32, tag="tmp",
                                               name="tmp")
                                nc.vector.scalar_tensor_tensor(
                                    tmp[:], ii[:], float(iv), bd[d][:],
                                    ALU.is_equal, ALU.mult)
                                nc.vector.tensor_add(tgt[:], acc[:], tmp[:])

                # ---- matmul sweep: W stationary, features moving ----
                for di in range(DTI):
                    wt = wp.tile([128, KT * 128], bf16, tag="w", name="wt")
                    # 4 parallel DMAs so the load spreads across queues
                    for q in range(4):
                        kspan = KT * 128 // 4
                        nc.sync.dma_start(
                            wt[:, q * kspan:(q + 1) * kspan],
                            w[di, :, q * kspan:(q + 1) * kspan])
                    ps = psump.tile([128, CHUNK], f32, tag="ps", name="ps")
                    for k in range(KT):
                        nc.tensor.matmul(ps[:], wt[:, k * 128:(k + 1) * 128],
                                         feat[k][:],
                                         start=(k == 0), stop=(k == KT - 1))
                    ot = outp.tile([128, CHUNK], f32, tag="o", name="ot")
                    nc.scalar.activation(ot[:], ps[:], AF.Gelu)
                    nc.sync.dma_start(
                        out[di * 128:(di + 1) * 128,
                            c * CHUNK:(c + 1) * CHUNK], ot[:])

    nc.compile()
    return nc


def _prep_weights(base_weight, spline_weight, spline_scaler):
    # Wcat rows: K = h (silu) then 1024 + j*1024 + h (spline plane j)
    wk = np.concatenate(
        [base_weight.T.astype(np.float32),
         (spline_weight * spline_scaler[..., None]).transpose(2, 1, 0)
         .reshape(NB * H, D)],
        axis=0)                                  # (9216, 4096)
    # -> [d_tile, kk, k*128 + dd] so each core-side W tile DMA is linear
    wt = wk.reshape(KT, 128, DTI, 128).transpose(2, 1, 0, 3) \
           .reshape(DTI, 128, KT * 128)
    return np.ascontiguousarray(wt.astype(ml_dtypes.bfloat16))


def kernel(x, base_weight, spline_weight, spline_scaler, _trace=False):
    if "nc" not in _NC_CACHE:
        _NC_CACHE["nc"] = _build_program()
    nc = _NC_CACHE["nc"]

    xf = np.asarray(x, np.float32).reshape(N_CORES * NTOK, H)
    wt = _prep_weights(np.asarray(base_weight, np.float32),
                       np.asarray(spline_weight, np.float32),
                       np.asarray(spline_scaler, np.float32))
    in_maps = []
    for c in range(N_CORES):
        xs = np.ascontiguousarray(xf[c * NTOK:(c + 1) * NTOK].T)  # (H, NTOK)
        in_maps.append({"xt": xs, "w": wt})

    res = run_bass_kernel_spmd(nc, in_maps, core_ids=list(range(N_CORES)),
                               trace=_trace)
    full = np.concatenate([res.results[c]["out"] for c in range(N_CORES)],
                          axis=1)               # (4096, 8192)
    out = np.ascontiguousarray(full.T).reshape(x.shape[0], x.shape[1], D)
    if _trace:
        kernel.last_exec_time_ns = res.exec_time_ns
        kernel.last_results = res
    return out.astype(np.float32, copy=False)


def measure_exec_ns(inputs, n=5):
    """Min wall time of repeated on-device executions (device-resident
    inputs, pre-staged donated output buffers) — upper bound on HW exec."""
    import time
    import jax
    from jax.sharding import Mesh, PartitionSpec, NamedSharding
    try:
        from jax.experimental.shard_map import shard_map
    except ImportError:
        from jax.shard_map import shard_map
    from concourse.bass2jax import (_bass_exec_p, install_neuronx_cc_hook,
                                    partition_id_tensor)

    if "nc" not in _NC_CACHE:
        _NC_CACHE["nc"] = _build_program()
    nc = _NC_CACHE["nc"]
    install_neuronx_cc_hook()

    pname = (nc.partition_id_tensor.name if nc.partition_id_tensor else None)
    in_names, out_names, out_avals, zero_outs = [], [], [], []
    for alloc in nc.m.functions[0].allocations:
        if not isinstance(alloc, mybir.MemoryLocationSet):
            continue
        name = alloc.memorylocations[0].name
        if alloc.kind == "ExternalInput":
            if name != pname:
                in_names.append(name)
        elif alloc.kind == "ExternalOutput":
            out_names.append(name)
            shape = tuple(alloc.tensor_shape)
            dtype = mybir.dt.np(alloc.dtype)
            out_avals.append(jax.core.ShapedArray(shape, dtype))
            zero_outs.append(np.zeros(shape, dtype))
    n_params = len(in_names)
    all_in = in_names + out_names
    if pname is not None:
        all_in = all_in + [pname]
    donate = tuple(range(n_params, n_params + len(out_names)))

    def _body(*args):
        operands = list(args)
        if pname is not None:
            operands.append(partition_id_tensor())
        outs = _bass_exec_p.bind(
            *operands, out_avals=tuple(out_avals), in_names=tuple(all_in),
            out_names=tuple(out_names), lowering_input_output_aliases=(),
            sim_require_finite=True, sim_require_nnan=True, nc=nc)
        return tuple(outs)

    xf = np.asarray(inputs["x"], np.float32).reshape(N_CORES * NTOK, H)
    wt = _prep_weights(np.asarray(inputs["base_weight"], np.float32),
                       np.asarray(inputs["spline_weight"], np.float32),
                       np.asarray(inputs["spline_scaler"], np.float32))
    per_core = {
        "xt": [np.ascontiguousarray(xf[c * NTOK:(c + 1) * NTOK].T)
               for c in range(N_CORES)],
        "w": [wt] * N_CORES,
    }
    devices = jax.devices()[:N_CORES]
    mesh = Mesh(np.asarray(devices), ("core",))
    sh = NamedSharding(mesh, PartitionSpec("core"))
    in_specs = (PartitionSpec("core"),) * (n_params + len(out_names))
    out_specs = (PartitionSpec("core"),) * len(out_names)
    fn = jax.jit(shard_map(_body, mesh=mesh, in_specs=in_specs,
                           out_specs=out_specs, check_rep=False),
                 donate_argnums=donate, keep_unused=True)
    concat_in = [jax.device_put(
        np.concatenate(per_core[name], axis=0), sh) for name in in_names]
    for a in concat_in:
        a.block_until_ready()
    times = []
    for trial in range(n):
        zeros = [jax.device_put(
            np.zeros((N_CORES * z.shape[0], *z.shape[1:]), z.dtype), sh)
            for z in zero_outs]
        for z in zeros:
            z.block_until_ready()
        t0 = time.perf_counter()
        outs = fn(*concat_in, *zeros)
        for o in outs:
            o.block_until_ready()
        dt_s = time.perf_counter() - t0
        if trial > 0:        # first call may include compile
            times.append(dt_s)
    return int(min(times) * 1e9)
